# revision 3
# baseline (speedup 1.0000x reference)
import numpy as np

# nn_Attention_352187318644: autoregressive decode, 32 steps over a 64-slot
# KV cache. B,H,D,S hardcoded from the problem spec. Head-sharded tensor
# parallelism: H=32 heads split 4-per-core across 8 NeuronCores (zero
# collectives; per-head weights and KV slices are independent). Falls back
# to a pure-numpy implementation if the device path is unavailable.
B, H, D, S = 128, 32, 128, 64
START_LEN = 32
SCALE = 0.125
NCORES = 8
HL = H // NCORES  # heads per core


def _decode_numpy(x, k, v, weight_q, weight_k, weight_v, weight_o):
    x = np.ascontiguousarray(x, dtype=np.float32)
    k = np.array(k, dtype=np.float32, copy=True)
    v = np.array(v, dtype=np.float32, copy=True)
    wq = np.ascontiguousarray(weight_q, dtype=np.float32)
    wk = np.ascontiguousarray(weight_k, dtype=np.float32)
    wv = np.ascontiguousarray(weight_v, dtype=np.float32)
    wo = np.ascontiguousarray(weight_o, dtype=np.float32)

    for g in range(START_LEN, S):
        q = np.matmul(x, wq)                                   # [B,H,1,D]
        k[:, :, g, :] = np.matmul(x, wk)[:, :, 0, :]
        v[:, :, g, :] = np.matmul(x, wv)[:, :, 0, :]
        attn = np.matmul(q, k.transpose(0, 1, 3, 2)) * SCALE   # [B,H,1,S]
        attn = attn - attn.max(axis=3, keepdims=True)
        np.exp(attn, out=attn)
        attn /= attn.sum(axis=3, keepdims=True)
        x = np.matmul(attn, v)                                 # [B,H,1,D]
        x = np.matmul(x, wo)
    return k, v, x


def _decode_neuron(x, k, v, weight_q, weight_k, weight_v, weight_o):
    """Head-sharded decode on 8 NeuronCores: pmap over the head axis."""
    import jax
    import jax.numpy as jnp

    devs = jax.devices()
    if len(devs) < NCORES or devs[0].platform == "cpu":
        raise RuntimeError("need 8 accelerator devices")

    def shard(a, batch_axis):
        # split head axis into [NCORES, HL] and move NCORES to front
        a = np.asarray(a, dtype=np.float32)
        if batch_axis == 1:   # x,k,v: [B, H, ...] -> [NCORES, B, HL, ...]
            s = a.reshape(a.shape[0], NCORES, HL, *a.shape[2:])
            return np.ascontiguousarray(np.moveaxis(s, 1, 0))
        else:                 # weights: [H, D, D] -> [NCORES, HL, D, D]
            return np.ascontiguousarray(a.reshape(NCORES, HL, *a.shape[1:]))

    xs = shard(x, 1)
    ks = shard(k, 1)
    vs = shard(v, 1)
    wqs = shard(weight_q, 0)
    wks = shard(weight_k, 0)
    wvs = shard(weight_v, 0)
    wos = shard(weight_o, 0)

    def decode(x, k, v, wq, wk, wv, wo):
        def step(carry, g):
            x, k, v = carry
            q = jnp.einsum('bhod,hde->bhoe', x, wq)
            k_new = jnp.einsum('bhod,hde->bhe', x, wk)
            v_new = jnp.einsum('bhod,hde->bhe', x, wv)
            k = jax.lax.dynamic_update_slice_in_dim(k, k_new[:, :, None, :], g, axis=2)
            v = jax.lax.dynamic_update_slice_in_dim(v, v_new[:, :, None, :], g, axis=2)
            attn = jnp.einsum('bhsd,bhod->bhos', k, q) * SCALE
            attn = jax.nn.softmax(attn, axis=3)
            o = jnp.einsum('bhos,bhsd->bhod', attn, v)
            x = jnp.einsum('bhod,hde->bhoe', o, wo)
            return (x, k, v), None

        (x, k, v), _ = jax.lax.scan(step, (x, k, v), jnp.arange(START_LEN, S))
        return x, k, v

    pdecode = jax.pmap(decode)
    xs, ks, vs = pdecode(xs, ks, vs, wqs, wks, wvs, wos)
    ks.block_until_ready()

    def unshard(a):  # [NCORES, B, HL, ...] -> [B, H, ...]
        a = np.asarray(a)
        return np.ascontiguousarray(np.moveaxis(a, 0, 1).reshape(
            a.shape[1], H, *a.shape[3:]))

    return unshard(ks), unshard(vs), unshard(xs)


def kernel(x, k, v, weight_q, weight_k, weight_v, weight_o):
    try:
        return _decode_neuron(x, k, v, weight_q, weight_k, weight_v, weight_o)
    except Exception:
        return _decode_numpy(x, k, v, weight_q, weight_k, weight_v, weight_o)


if __name__ == "__main__":
    rng = np.random.default_rng(0)
    args = {
        "x": rng.standard_normal((B, H, 1, D), dtype=np.float32),
        "k": rng.standard_normal((B, H, S, D), dtype=np.float32),
        "v": rng.standard_normal((B, H, S, D), dtype=np.float32),
        "weight_q": rng.standard_normal((H, D, D), dtype=np.float32) / np.sqrt(D),
        "weight_k": rng.standard_normal((H, D, D), dtype=np.float32) / np.sqrt(D),
        "weight_v": rng.standard_normal((H, D, D), dtype=np.float32) / np.sqrt(D),
        "weight_o": rng.standard_normal((H, D, D), dtype=np.float32) / np.sqrt(D),
    }
    ref = _decode_numpy(**args)
    out = kernel(**args)
    for name, a, e in zip("kvx", out, ref):
        print(name, float(np.abs(a - e).max() / np.abs(e).max()))
